# revision 1
# baseline (speedup 1.0000x reference)
"""GCN encoder (2-layer GCN with shared graph) on 8 Trainium2 NeuronCores.

Math (per gcn_conv, PyG GCNConv with edge weights, self-loops in edge list):
    deg[v]  = sum of w over edges (s -> v)            (in-degree, weighted)
    dinv    = deg ** -0.5                             (deg >= 1 always: self-loops)
    agg[d]  = dinv[d] * sum_s Wgt[s,d] * dinv[s] * h[s]
    out     = agg @ W + b
where Wgt[s,d] = total edge weight s->d:
    Wgt = count(edge_index) + I (self loops) + sigmoid(masked_y[:1024,:1024])
          (the sigmoid part only on the [0:1024) x [0:1024) block)

Sharding: core k owns destination-node blocks k and k+8 (128 nodes each,
256 total).  Each core holds Wgt[:, own-cols] ([2048, 256]) built from the
integer-count adjacency shard (host) + on-device sigmoid of its masked_y
column shard.  All float math (sigmoid, degrees, normalization, aggregation,
dense layers) runs on device.

Cross-core exchange (2 AllGathers of [256,128] bf16 each):
  AG1: x~_own = dinv_own * x_own        (layer-1 scaled inputs)
  AG2: g_own  = dinv_own * hidden_own   (layer-2 scaled inputs)
Each core scales its own rows (dinv_own is local: column sums of the own
Wgt shard), so no global dinv vector is ever needed.

Device pipeline per core (both layers share Wgt):
    Wgt  = adj_shard + sigmoid(my_shard)             (ACT + DVE)
    deg  = ones^T @ Wgt (16 matmuls, PSUM accum) -> sqrt -> 1/x
    x~_own -> AG1 -> xb tiles [2048,128] bf16
    aggT = sum_t xb_t^T @ Wgt_t                      (16 matmuls -> PSUM [128,256])
    rT   = W^T @ aggT + b (x) sqrt(deg)              (rank-1 bias trick)
    hidT = relu(dinv_bcast * rT);  g_own = transpose(dinv_bcast * hidT)
    g_own -> AG2 -> layer 2 (same shape) -> zT = dinv_bcast * r2T -> DRAM
"""

import numpy as np

N = 2048
HALF = 1024
F = 128          # IN_C == HID == 128
NCORES = 8
NT = 16          # 16 src-row tiles of 128
CPC = 256        # columns (dst nodes) per core

USE_BF16 = True

_COMPILED = {}


def _np_dt(use_bf16):
    if use_bf16:
        import ml_dtypes
        return np.dtype(ml_dtypes.bfloat16)
    return np.dtype(np.float32)


def _build_program(use_bf16):
    import concourse.bacc as bacc
    import concourse.tile as tile
    from concourse import mybir

    f32 = mybir.dt.float32
    DT = mybir.dt.bfloat16 if use_bf16 else f32
    npdt = _np_dt(use_bf16)
    AF = mybir.ActivationFunctionType
    MUL = mybir.AluOpType.mult

    nc = bacc.Bacc(
        "TRN2",
        target_bir_lowering=False,
        debug=False,
        enable_asserts=True,
        num_devices=NCORES,
    )

    # I/O (per-core shards; layouts pre-swizzled on host to [128, ...])
    adj_d = nc.dram_tensor("adj", [128, NT * CPC], DT, kind="ExternalInput")
    my_d = nc.dram_tensor("my", [128, 8 * F], DT, kind="ExternalInput")
    xo_d = nc.dram_tensor("xo", [128, 2 * F], f32, kind="ExternalInput")
    w1_d = nc.dram_tensor("w1", [F, F], DT, kind="ExternalInput")
    w2_d = nc.dram_tensor("w2", [F, F], DT, kind="ExternalInput")
    b1_d = nc.dram_tensor("b1", [1, F], DT, kind="ExternalInput")
    b2_d = nc.dram_tensor("b2", [1, F], DT, kind="ExternalInput")
    z_d = nc.dram_tensor("z", [128, CPC], f32, kind="ExternalOutput")

    ones_col_d = nc.inline_tensor(np.ones((128, 1), npdt), "ones_col")
    ones_row_d = nc.inline_tensor(np.ones((1, 128), np.float32), "ones_row")
    ones11_d = nc.inline_tensor(np.ones((1, 1), np.float32), "ones11")
    id128_d = nc.inline_tensor(np.eye(128).astype(npdt), "id128")

    rg = [list(range(NCORES))]

    with tile.TileContext(nc) as tc:
        with (
            tc.tile_pool(name="big", bufs=1) as big,
            tc.tile_pool(name="work", bufs=2) as work,
            tc.tile_pool(name="ps", bufs=1, space="PSUM") as ps,
            tc.tile_pool(name="dram", bufs=1, space="DRAM") as dram,
        ):
            # ---- loads ----
            # adj/my come in per-tile so the sigmoid-add + degree matmuls
            # pipeline behind the DMAs instead of waiting for the full 1MB.
            adj = big.tile([128, NT * CPC], DT, name="adj_sb")
            myt = big.tile([128, 8 * F], DT, name="my_sb")
            for q in range(2):
                nc.sync.dma_start(myt[:, 4 * F * q:4 * F * (q + 1)],
                                  my_d.ap()[:, 4 * F * q:4 * F * (q + 1)])
                for r in range(2):
                    c0 = CPC * (8 * q + 4 * r)
                    nc.sync.dma_start(adj[:, c0:c0 + 4 * CPC],
                                      adj_d.ap()[:, c0:c0 + 4 * CPC])
            xo = big.tile([128, 2 * F], f32, name="xo_sb")
            nc.sync.dma_start(xo[:], xo_d.ap())
            w1s = big.tile([F, F], DT, name="w1_sb")
            nc.sync.dma_start(w1s[:], w1_d.ap())
            w2s = big.tile([F, F], DT, name="w2_sb")
            nc.sync.dma_start(w2s[:], w2_d.ap())
            b1s = big.tile([1, F], DT, name="b1_sb")
            nc.sync.dma_start(b1s[:], b1_d.ap())
            b2s = big.tile([1, F], DT, name="b2_sb")
            nc.sync.dma_start(b2s[:], b2_d.ap())
            onec = big.tile([128, 1], DT, name="onec_sb")
            nc.sync.dma_start(onec[:], ones_col_d.ap())
            oner = big.tile([1, 128], f32, name="oner_sb")
            nc.sync.dma_start(oner[:], ones_row_d.ap())
            ones11 = big.tile([1, 1], f32, name="ones11_sb")
            nc.sync.dma_start(ones11[:], ones11_d.ap())
            id128s = big.tile([128, 128], DT, name="id128_sb")
            nc.sync.dma_start(id128s[:], id128_d.ap())

            # ---- Wgt = adj + sigmoid(masked_y shard) on dense region ----
            # src tiles t<8 (rows 0:1024), local cols 0:128 (own dense block)
            for t in range(8):
                sg = work.tile([128, F], DT, tag="sg")
                nc.scalar.activation(sg[:], myt[:, F * t:F * (t + 1)], AF.Sigmoid)
                nc.vector.tensor_add(
                    adj[:, CPC * t:CPC * t + F], adj[:, CPC * t:CPC * t + F], sg[:]
                )

            # ---- degree: deg = ones^T @ Wgt  (column sums over all 2048 srcs)
            ps_deg = ps.tile([1, CPC], f32, name="ps_deg")
            for t in range(NT):
                nc.tensor.matmul(
                    ps_deg[:], onec[:], adj[:, CPC * t:CPC * (t + 1)],
                    start=(t == 0), stop=(t == NT - 1),
                )
            sqd = big.tile([1, CPC], f32, name="sqd_sb")     # sqrt(deg) (own)
            nc.scalar.activation(sqd[:], ps_deg[:], AF.Sqrt)
            sqdb = big.tile([1, CPC], DT, name="sqdb_sb")    # bf16 copy for bias mm
            nc.vector.tensor_copy(sqdb[:], sqd[:])
            dinvr = big.tile([1, CPC], f32, name="dinvr_sb")  # deg^-1/2 (own)
            nc.vector.reciprocal(dinvr[:], sqd[:])

            # dinv broadcast [128, 256] (own cols, for per-column scaling)
            ps_bc = ps.tile([128, CPC], f32, name="ps_bc")
            nc.tensor.matmul(ps_bc[:], oner[:], dinvr[:], start=True, stop=True)
            dbc = big.tile([128, CPC], f32, name="dbc_sb")
            nc.vector.tensor_copy(dbc[:], ps_bc[:])
            dbc2 = big.tile([128, CPC], f32, name="dbc2_sb")  # dinv^2 bcast
            nc.vector.tensor_tensor(dbc2[:], dbc[:], dbc[:], op=MUL)

            # dinv_own as per-partition columns: dco[:, h] = dinv[128h + p]
            ps_dc = ps.tile([128, 2], f32, name="ps_dc")
            for h in range(2):
                nc.tensor.matmul(ps_dc[:, h:h + 1],
                                 dinvr[:, 128 * h:128 * (h + 1)],
                                 ones11[:], start=(h == 0), stop=(h == 1))
            dco = big.tile([128, 2], f32, name="dco_sb")
            nc.vector.tensor_copy(dco[:], ps_dc[:])

            # ---- AG1: x~_own = dinv_own * x_own ----
            xag = work.tile([128, 2 * F], DT, tag="xag")
            for h in range(2):
                nc.vector.tensor_scalar_mul(
                    xag[:, F * h:F * (h + 1)], xo[:, F * h:F * (h + 1)],
                    dco[:, h:h + 1],
                )
            ag1_in = dram.tile([CPC, F], DT, name="ag1_in")
            ag1_out = dram.tile([N, F], DT, name="ag1_out", addr_space="Shared")
            nc.scalar.dma_start(ag1_in[:].rearrange("(h p) c -> p h c", h=2), xag[:])
            nc.gpsimd.collective_compute(
                "AllGather", mybir.AluOpType.bypass,
                replica_groups=rg, ins=[ag1_in.opt()], outs=[ag1_out.opt()],
            )

            def load_gathered(ag_out, name):
                xb = big.tile([128, NT * F], DT, name=name)
                v = ag_out[:].rearrange("(r h p) c -> h p r c", h=2, p=128)
                # tile 0 lands first on its own sem so the first agg
                # matmul starts while the bulk of the gather still streams
                nc.scalar.dma_start(xb[:, 0:F], v[0][:, 0:1])
                nc.sync.dma_start(xb[:, F:8 * F], v[0][:, 1:8])
                nc.sync.dma_start(xb[:, 8 * F:16 * F], v[1])
                return xb

            def layer(xtiles, wsb, bsb, name):
                ps_agg = ps.tile([128, CPC], f32, name=f"ps_agg_{name}",
                                 tag="ps_agg")
                for t in range(NT):
                    nc.tensor.matmul(
                        ps_agg[:], xtiles[:, F * t:F * (t + 1)],
                        adj[:, CPC * t:CPC * (t + 1)],
                        start=(t == 0), stop=(t == NT - 1),
                    )
                aggs = work.tile([128, CPC], DT, tag="aggs")
                nc.vector.tensor_copy(aggs[:], ps_agg[:])
                ps_r = ps.tile([128, CPC], f32, name=f"ps_r_{name}", tag="ps_r")
                nc.tensor.matmul(ps_r[:], wsb[:], aggs[:], start=True, stop=False)
                nc.tensor.matmul(ps_r[:], bsb[:], sqdb[:], start=False, stop=True)
                return ps_r

            # ---- layer 1 ----
            xb1 = load_gathered(ag1_out, "xb1_sb")
            ps_r1 = layer(xb1, w1s, b1s, "l1")
            # dinv*relu(dinv*r) == dinv^2*relu(r)  (dinv > 0 since deg >= 1)
            r1r = work.tile([128, CPC], f32, tag="m1")
            nc.scalar.activation(r1r[:], ps_r1[:], AF.Relu)
            gT = work.tile([128, CPC], DT, tag="gT")
            nc.vector.tensor_tensor(gT[:], r1r[:], dbc2[:], op=MUL)

            # transpose gT -> g [256, 128] (rows = own dst nodes)
            g01 = work.tile([128, 2 * 128], DT, tag="g01")
            for h in range(2):
                ps_g = ps.tile([128, 128], DT, name=f"ps_g{h}", tag="ps_g", bufs=2)
                nc.tensor.transpose(ps_g[:], gT[:, 128 * h:128 * (h + 1)],
                                    id128s[:])
                nc.vector.tensor_copy(g01[:, 128 * h:128 * (h + 1)], ps_g[:])

            # ---- AG2 ----
            ag2_in = dram.tile([CPC, F], DT, name="ag2_in")
            ag2_out = dram.tile([N, F], DT, name="ag2_out", addr_space="Shared")
            nc.scalar.dma_start(ag2_in[:].rearrange("(h p) c -> p h c", h=2), g01[:])
            nc.gpsimd.collective_compute(
                "AllGather", mybir.AluOpType.bypass,
                replica_groups=rg, ins=[ag2_in.opt()], outs=[ag2_out.opt()],
            )

            # ---- layer 2 ----
            xb2 = load_gathered(ag2_out, "xb2_sb")
            ps_r2 = layer(xb2, w2s, b2s, "l2")
            zT = work.tile([128, CPC], f32, tag="zT")
            nc.vector.tensor_tensor(zT[:], ps_r2[:], dbc[:], op=MUL)
            nc.scalar.dma_start(z_d.ap(), zT[:])

    nc.compile()
    return nc


def _host_prep(x, masked_y, W1, b1, Wmu, bmu, Wls, bls, edge_index, use_bf16):
    npdt = _np_dt(use_bf16)
    src = edge_index[0].astype(np.int64)
    dst = edge_index[1].astype(np.int64)

    A = np.zeros((N, N), np.float32)
    np.add.at(A, (src, dst), 1.0)
    idx = np.arange(N)
    A[idx, idx] += 1.0

    W2 = np.concatenate([Wmu, Wls], axis=1).astype(npdt)
    b1r = np.ascontiguousarray(b1.reshape(1, F)).astype(npdt)
    b2r = np.concatenate([bmu, bls]).reshape(1, F).astype(npdt)
    W1c = np.ascontiguousarray(W1).astype(npdt)

    in_maps = []
    for k in range(NCORES):
        cols = np.r_[128 * k:128 * k + 128, HALF + 128 * k:HALF + 128 * k + 128]
        adj_k = A[:, cols]  # [2048, 256]
        adj_sw = np.ascontiguousarray(
            adj_k.reshape(NT, 128, CPC).transpose(1, 0, 2).reshape(128, NT * CPC)
        ).astype(npdt)
        my_k = masked_y[:HALF, F * k:F * (k + 1)]  # [1024, 128]
        my_sw = np.ascontiguousarray(
            my_k.reshape(8, 128, F).transpose(1, 0, 2).reshape(128, 8 * F)
        ).astype(npdt)
        xo_k = x[cols]  # [256, 128] own rows
        xo_sw = np.ascontiguousarray(
            xo_k.reshape(2, 128, F).transpose(1, 0, 2).reshape(128, 2 * F)
        ).astype(np.float32)
        in_maps.append({
            "adj": adj_sw,
            "my": my_sw,
            "xo": xo_sw,
            "w1": W1c,
            "w2": W2,
            "b1": b1r,
            "b2": b2r,
        })
    return in_maps


def _assemble(results):
    zfull = np.empty((N, F), np.float32)
    for k in range(NCORES):
        zk = results[k]["z"]  # [128, 256]
        zfull[128 * k:128 * (k + 1)] = zk[:, 0:128].T
        zfull[HALF + 128 * k:HALF + 128 * (k + 1)] = zk[:, 128:256].T
    return zfull[:, :F // 2].copy(), zfull[:, F // 2:].copy()


def _make_runner(nc):
    """Cached shard_map runner (mirror of bass2jax.run_bass_via_pjrt's
    multi-core branch, minus donation so the jitted fn is reusable)."""
    import jax
    from jax.sharding import Mesh, PartitionSpec
    from jax.experimental.shard_map import shard_map
    from concourse import bass2jax, mybir

    bass2jax.install_neuronx_cc_hook()

    partition_name = (nc.partition_id_tensor.name
                      if nc.partition_id_tensor else None)
    in_names, out_names, out_avals, zero_outs = [], [], [], []
    for alloc in nc.m.functions[0].allocations:
        if not isinstance(alloc, mybir.MemoryLocationSet):
            continue
        name = alloc.memorylocations[0].name
        if alloc.kind == "ExternalInput":
            if name != partition_name:
                in_names.append(name)
        elif alloc.kind == "ExternalOutput":
            out_names.append(name)
            shape = tuple(alloc.tensor_shape)
            dtype = mybir.dt.np(alloc.dtype)
            out_avals.append(jax.core.ShapedArray(shape, dtype))
            zero_outs.append(np.zeros(shape, dtype))
    n_params = len(in_names)
    all_names = in_names + out_names
    if partition_name is not None:
        all_names = all_names + [partition_name]

    def _body(*args):
        operands = list(args)
        if partition_name is not None:
            operands.append(bass2jax.partition_id_tensor())
        outs = bass2jax._bass_exec_p.bind(
            *operands,
            out_avals=tuple(out_avals),
            in_names=tuple(all_names),
            out_names=tuple(out_names),
            lowering_input_output_aliases=(),
            sim_require_finite=True,
            sim_require_nnan=True,
            nc=nc,
        )
        return tuple(outs)

    devices = jax.devices()[:NCORES]
    mesh = Mesh(np.asarray(devices), ("core",))
    sharded = jax.jit(
        shard_map(
            _body, mesh=mesh,
            in_specs=(PartitionSpec("core"),) * (n_params + len(out_names)),
            out_specs=(PartitionSpec("core"),) * len(out_names),
            check_rep=False,
        ),
        keep_unused=True,
    )
    sharding = jax.sharding.NamedSharding(mesh, PartitionSpec("core"))

    def run(in_maps):
        from concourse import bass2jax as b2j
        results = b2j.run_bass_via_pjrt(nc, in_maps, n_cores=NCORES)
        return results

    return run


def kernel(x, masked_y, W1, b1, Wmu, bmu, Wls, bls, edge_index,
           _trace=False, _warm=True):
    use_bf16 = USE_BF16
    if "nc" not in _COMPILED or _COMPILED.get("bf16") != use_bf16:
        _COMPILED["nc"] = _build_program(use_bf16)
        _COMPILED["bf16"] = use_bf16
        _COMPILED["run"] = _make_runner(_COMPILED["nc"])

    in_maps = _host_prep(
        np.asarray(x, np.float32), np.asarray(masked_y, np.float32),
        np.asarray(W1, np.float32), np.asarray(b1, np.float32),
        np.asarray(Wmu, np.float32), np.asarray(bmu, np.float32),
        np.asarray(Wls, np.float32), np.asarray(bls, np.float32),
        np.asarray(edge_index), use_bf16,
    )
    run = _COMPILED["run"]
    if _warm and not _COMPILED.get("warmed"):
        run(in_maps)  # first call pays NEFF load on every core
        _COMPILED["warmed"] = True
    if _trace:
        import tempfile
        try:
            from antenv import axon_hooks
            hook = axon_hooks.get_axon_ntff_profile_hook()
        except ImportError:
            hook = None
        if hook is None:
            results = run(in_maps)
        else:
            neff_dir = tempfile.mkdtemp()
            with hook(neff_dir, list(range(NCORES))):
                results = run(in_maps)
            _COMPILED["ntff_dir"] = neff_dir
            try:
                import gauge.profiler
                from concourse._compat import FishPath
                from concourse.bass_utils import _process_ntff_profile
                profile = gauge.profiler.Profile(
                    profile_path=FishPath(neff_dir), kernel_dev_mode=True,
                    profile_on_exit=False, bass_kernel=_COMPILED["nc"].m,
                    offline_processing=True, fname="*_body*",
                )
                r = _process_ntff_profile(
                    profile, neff_dir, _COMPILED["nc"], list(range(NCORES)),
                    list(range(NCORES)), False, {}, trace_events=False,
                )
                _COMPILED["exec_time_ns"] = r.exec_time_ns
                _COMPILED["mean_exec_time_ns"] = r.mean_exec_time_ns
            except Exception as e:
                _COMPILED["exec_time_ns"] = None
                _COMPILED["trace_err"] = repr(e)
    else:
        results = run(in_maps)
    return _assemble(results)



# revision 13
# speedup vs baseline: 1.2046x; 1.2046x over previous
"""GCN encoder (2-layer GCN, shared graph) on 8 Trainium2 NeuronCores.

Collective-free design: a single tiny AllGather on this platform costs
75-140us of max-span (cross-core dispatch skew absorbed at the first
rendezvous), while an 8MB HBM load costs ~21us.  So every core gets the
FULL graph and computes layer 1 for ALL 2048 nodes redundantly; layer 2
is computed only for the core's own 256 destination columns.  No
cross-core sync at all -> per-core span is immune to launch skew.

Math (PyG GCNConv, self-loops in the edge list):
    Wgt  = count(edge_index) + I + sigmoid(masked_y[:1024,:1024]) block
    deg  = colsum(Wgt); dinv = deg^-1/2
    h1   = relu(dinv[d] * sum_s Wgt[s,d] dinv[s] x[s] @ W1 + b1)
    z    = dinv[c] * sum_s Wgt[s,c] dinv[s] h1[s] @ [Wmu|Wls] + b2

Precision: adjacency in fp8e4 (integer counts exact; sigmoid quantization
averages out), aggregation matmuls in fp8 DoubleRow mode (2 k-tiles per
pass), with an fp8 residual term for the scaled features x~ (the dominant
quantization error; the residual brings it from ~1.4% to ~0.25%).  The
dense layers run in bf16.  Scales: x~ carries 16x (fp8 subnormal
avoidance, compensated in W1/16 and W1/256 for the residual), h2 carries
256x via W2*256 (compensated in the final dinv_own/256).

deg is computed ON DEVICE as column sums of the fp8 adjacency AFTER the
sigmoid add - exactly consistent with the weights the aggregation uses.
The host supplies only the integer column sums for the sigmoid-free rows
(tiles 8..15) and the per-core own-column slices (adjown/myown), keeping
the SPMD program identical across cores.
"""

import numpy as np

N = 2048
HALF = 1024
F = 128          # IN_C == HID == 128
NT = 16          # 16 src-row tiles of 128
NCORES = 8
CH = 512         # column chunk (one PSUM bank of f32)
NCH = 4

_COMPILED = {}


def _np_f8():
    import ml_dtypes
    return np.dtype(ml_dtypes.float8_e4m3)


def _np_bf():
    import ml_dtypes
    return np.dtype(ml_dtypes.bfloat16)


def _build_program():
    import concourse.bacc as bacc
    import concourse.tile as tile
    from concourse import mybir

    f32 = mybir.dt.float32
    f32r = mybir.dt.float32r
    bf16 = mybir.dt.bfloat16
    f8 = mybir.dt.float8e4
    AF = mybir.ActivationFunctionType
    MUL = mybir.AluOpType.mult
    ADD = mybir.AluOpType.add
    DR = mybir.MatmulPerfMode.DoubleRow
    npf8 = _np_f8()

    nc = bacc.Bacc(
        "TRN2",
        target_bir_lowering=False,
        debug=False,
        enable_asserts=True,
        num_devices=NCORES,
    )

    # ---- I/O ----
    # adj8: [2048, 2048] int adjacency (+self loops), fp8, swizzled to
    # [128, chunk(4), tile(16), 512] (column-chunk-major so one DMA
    # delivers a full 512-column chunk across all 16 source tiles).
    adj8_d = nc.dram_tensor("adj8", [128, NCH * NT * CH], f8, kind="ExternalInput")
    my8_d = nc.dram_tensor("my8", [128, 8 * HALF], f8, kind="ExternalInput")
    xb_d = nc.dram_tensor("xb", [128, NT * F], bf16, kind="ExternalInput")
    adjo_d = nc.dram_tensor("adjo", [128, NT * 256], f8, kind="ExternalInput")
    myo_d = nc.dram_tensor("myo", [128, 8 * F], f8, kind="ExternalInput")
    w1a_d = nc.dram_tensor("w1a", [F, F], bf16, kind="ExternalInput")   # W1/16
    w1b_d = nc.dram_tensor("w1b", [F, F], bf16, kind="ExternalInput")   # W1/256
    w2_d = nc.dram_tensor("w2", [F, F], bf16, kind="ExternalInput")     # W2cat*256
    b1r_d = nc.dram_tensor("b1r", [1, F], bf16, kind="ExternalInput")
    b2b_d = nc.dram_tensor("b2b", [128, F], f32, kind="ExternalInput")  # bcast b2
    ci16_d = nc.dram_tensor("ci16", [16, 128], f32, kind="ExternalInput")
    z_d = nc.dram_tensor("z", [128, 256], f32, kind="ExternalOutput")

    import ml_dtypes
    onecb_d = nc.inline_tensor(np.ones((128, 1), ml_dtypes.bfloat16), "onecb")
    # [128, 2, 16] so the DoubleRow k-tile stride is 16B-aligned; col 0 used.
    onec_d = nc.inline_tensor(np.ones((128, 32), npf8), "onec8")
    oner_d = nc.inline_tensor(np.ones((1, 128), np.float32), "oner")
    id16_d = nc.inline_tensor(np.eye(16).astype(np.float32), "id16")
    id128_d = nc.inline_tensor(np.eye(128).astype(np.float32), "id128")

    with tile.TileContext(nc) as tc:
        with (
            tc.tile_pool(name="big", bufs=1) as big,
            tc.tile_pool(name="work", bufs=2) as work,
            tc.tile_pool(name="ps", bufs=1, space="PSUM") as ps,
        ):
            # ================= DMA loads =================
            # sync queue: my8 (4 DMAs, sigmoid path wants tiles early),
            # then adj8 (4 column-chunk DMAs).
            my8 = big.tile([128, 8, HALF], f8, name="my8_sb")
            for q in range(4):
                nc.sync.dma_start(
                    my8[:, 2 * q:2 * q + 2, :],
                    my8_d.ap()[:, 2 * HALF * q:2 * HALF * (q + 1)])
            adj = big.tile([128, NCH, NT, CH], f8, name="adj_sb")
            for c in range(NCH):
                nc.sync.dma_start(
                    adj[:, c, :, :],
                    adj8_d.ap()[:, NT * CH * c:NT * CH * (c + 1)])
            # gpsimd queue: everything else, needed-first order.
            xb = big.tile([128, NT, F], bf16, name="xb_sb")
            nc.gpsimd.dma_start(xb[:], xb_d.ap())
            onecb = big.tile([128, 1], bf16, name="onecb_sb")
            nc.gpsimd.dma_start(onecb[:], onecb_d.ap())
            onec8 = big.tile([128, 2, 16], f8, name="onec8_sb")
            nc.gpsimd.dma_start(onec8[:], onec_d.ap())
            oner = big.tile([1, 128], f32, name="oner_sb")
            nc.gpsimd.dma_start(oner[:], oner_d.ap())
            id16 = big.tile([16, 16], f32, name="id16_sb")
            nc.gpsimd.dma_start(id16[:], id16_d.ap())
            id128 = big.tile([128, 128], f32, name="id128_sb")
            nc.gpsimd.dma_start(id128[:], id128_d.ap())
            ci16 = big.tile([16, 128], f32, name="ci16_sb")
            nc.gpsimd.dma_start(ci16[:], ci16_d.ap())
            w1a = big.tile([F, F], bf16, name="w1a_sb")
            nc.gpsimd.dma_start(w1a[:], w1a_d.ap())
            w1b = big.tile([F, F], bf16, name="w1b_sb")
            nc.gpsimd.dma_start(w1b[:], w1b_d.ap())
            w2 = big.tile([F, F], bf16, name="w2_sb")
            nc.gpsimd.dma_start(w2[:], w2_d.ap())
            b1r = big.tile([1, F], bf16, name="b1r_sb")
            nc.gpsimd.dma_start(b1r[:], b1r_d.ap())
            myo = big.tile([128, 8, F], f8, name="myo_sb")
            nc.gpsimd.dma_start(myo[:], myo_d.ap())
            adjo = big.tile([128, NT, 256], f8, name="adjo_sb")
            nc.gpsimd.dma_start(adjo[:], adjo_d.ap())
            b2b = big.tile([128, F], f32, name="b2b_sb")
            nc.gpsimd.dma_start(b2b[:], b2b_d.ap())

            # ============ sigmoid add (fp8) + bf16 accumulator ============
            # S8_t = sigmoid(my tile t) in fp8; add into adj chunks 0/1 and
            # into the bf16 accumulator for the degree column sums.
            sacc = big.tile([128, HALF], bf16, name="sacc_sb")
            for t in range(8):
                s8 = work.tile([128, HALF], f8, tag="s8", name="s8")
                nc.scalar.activation(s8[:], my8[:, t, :], AF.Sigmoid)
                for c in range(2):
                    nc.vector.tensor_tensor(
                        adj[:, c, t, :], adj[:, c, t, :],
                        s8[:, CH * c:CH * (c + 1)], op=ADD)
                if t == 0:
                    nc.vector.tensor_copy(sacc[:], s8[:])
                else:
                    nc.vector.tensor_tensor(sacc[:], sacc[:], s8[:], op=ADD)

            # ============ degree ============
            # deg rows 0:8 = int colsums + sigmoid colsums (PE over sacc);
            # rows 8:16 = pure int colsums from host.
            dg16 = big.tile([16, 128], f32, name="dg16_sb")
            sigrow = big.tile([1, HALF], f32, name="sigrow_sb")
            for c in range(2):
                ps_sg = ps.tile([1, CH], f32, tag="small", name="ps_sg")
                nc.tensor.matmul(ps_sg[:], onecb[:],
                                 sacc[:, CH * c:CH * (c + 1)],
                                 start=True, stop=True)
                nc.vector.tensor_copy(sigrow[:, CH * c:CH * (c + 1)], ps_sg[:])
            sig16 = big.tile([8, 128], f32, name="sig16_sb")
            nc.gpsimd.dma_start(sig16[:], sigrow[:])
            nc.vector.tensor_copy(dg16[:], ci16[:])
            nc.vector.tensor_tensor(dg16[0:8, :], dg16[0:8, :], sig16[:],
                                    op=ADD)

            # pm path: dinv as per-partition scalars [128, 16]
            ps_T = ps.tile([128, 16], f32, tag="small", name="ps_T")
            nc.tensor.transpose(ps_T[:], dg16[:], id16[:])
            sqd_pm = big.tile([128, 16], f32, name="sqd_pm")
            nc.scalar.activation(sqd_pm[:], ps_T[:], AF.Sqrt)
            dinv_pm = big.tile([128, 16], f32, name="dinv_pm")
            nc.vector.reciprocal(dinv_pm[:], sqd_pm[:])
            dinv256 = big.tile([128, 16], f32, name="dinv256")
            nc.vector.tensor_scalar_mul(dinv256[:], dinv_pm[:], 256.0)

            # ============ x~ = dinv[s]*x*16 in fp8 + residual ============
            xq = big.tile([128, NT, F], f8, name="xq_sb")
            rq = big.tile([128, NT, F], f8, name="rq_sb")
            for t in range(NT):
                xs = work.tile([128, F], bf16, tag="xs", name="xs")
                # xs = x * dinv * 256
                nc.vector.tensor_scalar_mul(
                    xs[:], xb[:, t, :], dinv256[:, t:t + 1])
                # xq = xs/16 (fp8 rounding)
                nc.scalar.activation(xq[:, t, :], xs[:], AF.Copy, scale=0.0625)
                # rq = xs - 16*xq = 16 * (x~*16 - xq)
                nc.vector.scalar_tensor_tensor(
                    rq[:, t, :], xq[:, t, :], -16.0, xs[:],
                    op0=MUL, op1=ADD)

            # own columns: sigmoid(myown) into adjo tiles 0..7, block 0.
            for t in range(8):
                so = work.tile([128, F], f8, tag="so", name="so")
                nc.scalar.activation(so[:], myo[:, t, :], AF.Sigmoid)
                nc.vector.tensor_tensor(
                    adjo[:, t, 0:F], adjo[:, t, 0:F], so[:], op=ADD)

            # row path: dinv^2 broadcast + sqrt(deg) row for the bias trick
            dsq_pm = big.tile([128, 16], f32, name="dsq_pm")
            nc.vector.tensor_tensor(dsq_pm[:], dinv_pm[:], dinv_pm[:], op=MUL)
            ps_tq = ps.tile([16, 128], f32, tag="small", name="ps_tq")
            nc.tensor.transpose(ps_tq[:], dsq_pm[:], id128[:])
            ds16 = big.tile([16, 128], f32, name="ds16_sb")
            nc.vector.tensor_copy(ds16[:], ps_tq[:])
            dinv2_row = big.tile([1, N], f32, name="dinv2_row")
            nc.gpsimd.dma_start(dinv2_row[:], ds16[:])
            deg_row = big.tile([1, N], f32, name="deg_row")
            nc.gpsimd.dma_start(deg_row[:], dg16[:])
            sqdb_row = big.tile([1, N], bf16, name="sqdb_row")
            nc.scalar.activation(sqdb_row[:], deg_row[:], AF.Sqrt)
            dinv2b = big.tile([128, N], f32, name="dinv2b_sb")
            for c in range(NCH):
                ps_bc = ps.tile([128, CH], f32, tag="small", name="ps_bc")
                nc.tensor.matmul(
                    ps_bc[:], oner[:].bitcast(f32r),
                    dinv2_row[:, CH * c:CH * (c + 1)].bitcast(f32r),
                    start=True, stop=True)
                nc.vector.tensor_copy(dinv2b[:, CH * c:CH * (c + 1)], ps_bc[:])

            # ============ main pipeline per column chunk ============
            x2T = big.tile([128, N], bf16, name="x2T_sb")
            h2sb = big.tile([128, NT, F], bf16, name="h2sb")
            psA2a = ps.tile([128, F], f32, tag="a2a", name="psA2a")
            psA2b = ps.tile([128, F], f32, tag="a2b", name="psA2b")
            dinv_own = big.tile([128, 2], f32, name="dinv_own")

            def chunk(c):
                # L1 aggregation: A1a = sum_s adj[s, cols_c] * xq[s, :]
                # (DoubleRow: two 128-row k-tiles per pass), A1b residual.
                psA1a = ps.tile([128, CH], f32, tag="a1a", name="psA1a")
                psA1b = ps.tile([128, CH], f32, tag="a1b", name="psA1b")
                for p in range(8):
                    nc.tensor.matmul(
                        psA1a[:], xq[:, 2 * p:2 * p + 2, :],
                        adj[:, c, 2 * p:2 * p + 2, :],
                        start=(p == 0), stop=(p == 7), perf_mode=DR)
                for p in range(8):
                    nc.tensor.matmul(
                        psA1b[:], rq[:, 2 * p:2 * p + 2, :],
                        adj[:, c, 2 * p:2 * p + 2, :],
                        start=(p == 0), stop=(p == 7), perf_mode=DR)
                a1a = work.tile([128, CH], bf16, tag="a1a_sb", name="a1a")
                nc.vector.tensor_copy(a1a[:], psA1a[:])
                a1b = work.tile([128, CH], bf16, tag="a1b_sb", name="a1b")
                nc.vector.tensor_copy(a1b[:], psA1b[:])
                # R1 = W1^T A1 (scales folded) + b1 (x) sqrt(deg)
                psR1 = ps.tile([128, CH], f32, tag="r1", name="psR1")
                nc.tensor.matmul(psR1[:], w1a[:], a1a[:], start=True, stop=False)
                nc.tensor.matmul(psR1[:], w1b[:], a1b[:], start=False, stop=False)
                nc.tensor.matmul(psR1[:], b1r[:],
                                 sqdb_row[:, CH * c:CH * (c + 1)],
                                 start=False, stop=True)
                # x2~^T = dinv^2[d] * relu(R1)  (bf16, feat-major)
                x2a = work.tile([128, CH], bf16, tag="x2a", name="x2a")
                nc.scalar.activation(x2a[:], psR1[:], AF.Relu)
                nc.vector.tensor_tensor(
                    x2T[:, CH * c:CH * (c + 1)], x2a[:],
                    dinv2b[:, CH * c:CH * (c + 1)], op=MUL)
                # h2 = x2~ @ (W2*256): node-major tiles, evac to fp8
                for tt in range(4 * c, 4 * c + 4):
                    psH2 = ps.tile([128, F], f32, tag="h2", name="psH2")
                    nc.tensor.matmul(psH2[:], x2T[:, F * tt:F * (tt + 1)],
                                     w2[:], start=True, stop=True)
                    nc.scalar.activation(h2sb[:, tt, :], psH2[:], AF.Copy)
                # L2 aggregation over own 256 cols (fp8 DoubleRow),
                # node-major output: A2[c_own, f] accumulated over pairs.
                for p in range(2 * c, 2 * c + 2):
                    h2q = work.tile([128, 2, F], f8, tag="h2q", name="h2q")
                    nc.vector.tensor_copy(h2q[:], h2sb[:, 2 * p:2 * p + 2, :])
                    nc.tensor.matmul(
                        psA2a[:], adjo[:, 2 * p:2 * p + 2, 0:F], h2q[:],
                        start=(p == 0), stop=(p == 7), perf_mode=DR)
                    nc.tensor.matmul(
                        psA2b[:], adjo[:, 2 * p:2 * p + 2, F:256], h2q[:],
                        start=(p == 0), stop=(p == 7), perf_mode=DR)

            chunk(0)
            chunk(1)

            # ============ own-column degree (placed here so the PE never
            # stalls on it: its deps are long since ready) ============
            ps_do = ps.tile([1, 256], f32, tag="small", name="ps_do")
            for p in range(8):
                nc.tensor.matmul(
                    ps_do[:], onec8[:, :, 0:1],
                    adjo[:, 2 * p:2 * p + 2, :],
                    start=(p == 0), stop=(p == 7), perf_mode=DR)
            deg_own = big.tile([1, 256], f32, name="deg_own")
            nc.vector.tensor_copy(deg_own[:], ps_do[:])
            do2 = big.tile([2, 128], f32, name="do2_sb")
            nc.gpsimd.dma_start(do2[:], deg_own[:])
            ps_to = ps.tile([128, 2], f32, tag="small", name="ps_to")
            nc.tensor.transpose(ps_to[:], do2[:], id16[0:2, 0:2])
            sq_own = big.tile([128, 2], f32, name="sq_own")
            nc.scalar.activation(sq_own[:], ps_to[:], AF.Sqrt)
            nc.vector.reciprocal(dinv_own[:], sq_own[:])
            # fold the h2 256x scale into the final dinv_own
            nc.vector.tensor_scalar_mul(dinv_own[:], dinv_own[:], 1.0 / 256.0)

            chunk(2)
            chunk(3)

            # ============ z = dinv_own * A2 + b2 (node-major) ============
            # zs[:, 0:128] = own block 0 (rows 128k..), zs[:, 128:256] =
            # own block 1 (rows 1024+128k..); partitions are nodes.
            zs = big.tile([128, 256], f32, name="zs_sb")
            nc.vector.scalar_tensor_tensor(
                zs[:, 0:F], psA2a[:], dinv_own[:, 0:1], b2b[:],
                op0=MUL, op1=ADD)
            nc.vector.scalar_tensor_tensor(
                zs[:, F:256], psA2b[:], dinv_own[:, 1:2], b2b[:],
                op0=MUL, op1=ADD)
            nc.scalar.dma_start(z_d.ap(), zs[:])

    nc.compile()
    return nc


def _host_prep(x, masked_y, W1, b1, Wmu, bmu, Wls, bls, edge_index):
    npf8 = _np_f8()
    npbf = _np_bf()
    src = edge_index[0].astype(np.int64)
    dst = edge_index[1].astype(np.int64)

    A = np.zeros((N, N), np.float32)
    np.add.at(A, (src, dst), 1.0)
    idx = np.arange(N)
    A[idx, idx] += 1.0

    # shared tensors (identical on every core)
    adj_sw = A.reshape(NT, 128, N).transpose(1, 0, 2)           # [128,16,2048]
    adj8 = np.ascontiguousarray(
        adj_sw.reshape(128, NT, NCH, CH).transpose(0, 2, 1, 3)
        .reshape(128, NCH * NT * CH)).astype(npf8)
    my8 = np.ascontiguousarray(
        masked_y[:HALF, :HALF].reshape(8, 128, HALF).transpose(1, 0, 2)
        .reshape(128, 8 * HALF)).astype(npf8)
    xb = np.ascontiguousarray(
        x.reshape(NT, 128, F).transpose(1, 0, 2).reshape(128, NT * F)
    ).astype(npbf)
    w1a = (np.ascontiguousarray(W1) / 16.0).astype(npbf)
    w1b = (np.ascontiguousarray(W1) / 256.0).astype(npbf)
    w2 = (np.concatenate([Wmu, Wls], axis=1) * 256.0).astype(npbf)
    b1r = b1.reshape(1, F).astype(npbf)
    b2 = np.concatenate([bmu, bls]).astype(np.float32)
    b2b = np.ascontiguousarray(np.broadcast_to(b2[None, :], (128, F))).astype(
        np.float32)
    ci = A.sum(axis=0)                                          # exact ints
    ci16 = np.ascontiguousarray(ci.reshape(16, 128)).astype(np.float32)

    in_maps = []
    for k in range(NCORES):
        cols = np.r_[128 * k:128 * k + 128, HALF + 128 * k:HALF + 128 * k + 128]
        adjo = np.ascontiguousarray(
            A[:, cols].reshape(NT, 128, 256).transpose(1, 0, 2)
            .reshape(128, NT * 256)).astype(npf8)
        myo = np.ascontiguousarray(
            masked_y[:HALF, 128 * k:128 * (k + 1)].reshape(8, 128, F)
            .transpose(1, 0, 2).reshape(128, 8 * F)).astype(npf8)
        in_maps.append({
            "adj8": adj8, "my8": my8, "xb": xb, "adjo": adjo, "myo": myo,
            "w1a": w1a, "w1b": w1b, "w2": w2, "b1r": b1r, "b2b": b2b,
            "ci16": ci16,
        })
    return in_maps


def _assemble(results):
    zfull = np.empty((N, F), np.float32)
    for k in range(NCORES):
        zk = results[k]["z"]  # [128 own nodes, 256 = block0 feat | block1 feat]
        zfull[128 * k:128 * (k + 1)] = zk[:, 0:128]
        zfull[HALF + 128 * k:HALF + 128 * (k + 1)] = zk[:, 128:256]
    return zfull[:, :F // 2].copy(), zfull[:, F // 2:].copy()


def _make_runner(nc):
    """Cached shard_map runner (mirror of bass2jax.run_bass_via_pjrt's
    multi-core branch, minus donation so the jitted fn is reusable)."""
    from concourse import bass2jax

    bass2jax.install_neuronx_cc_hook()

    def run(in_maps):
        return bass2jax.run_bass_via_pjrt(nc, in_maps, n_cores=NCORES)

    return run


def kernel(x, masked_y, W1, b1, Wmu, bmu, Wls, bls, edge_index,
           _trace=False, _warm=True):
    if "nc" not in _COMPILED:
        _COMPILED["nc"] = _build_program()
        _COMPILED["run"] = _make_runner(_COMPILED["nc"])

    in_maps = _host_prep(
        np.asarray(x, np.float32), np.asarray(masked_y, np.float32),
        np.asarray(W1, np.float32), np.asarray(b1, np.float32),
        np.asarray(Wmu, np.float32), np.asarray(bmu, np.float32),
        np.asarray(Wls, np.float32), np.asarray(bls, np.float32),
        np.asarray(edge_index),
    )
    run = _COMPILED["run"]
    if _warm and not _COMPILED.get("warmed"):
        run(in_maps)  # first call pays NEFF load on every core
        _COMPILED["warmed"] = True
    if _trace:
        import tempfile
        try:
            from antenv import axon_hooks
            hook = axon_hooks.get_axon_ntff_profile_hook()
        except ImportError:
            hook = None
        if hook is None:
            results = run(in_maps)
        else:
            neff_dir = tempfile.mkdtemp()
            with hook(neff_dir, list(range(NCORES))):
                results = run(in_maps)
            _COMPILED["ntff_dir"] = neff_dir
            try:
                import gauge.profiler
                from concourse._compat import FishPath
                from concourse.bass_utils import _process_ntff_profile
                profile = gauge.profiler.Profile(
                    profile_path=FishPath(neff_dir), kernel_dev_mode=True,
                    profile_on_exit=False, bass_kernel=_COMPILED["nc"].m,
                    offline_processing=True, fname="*_body*",
                )
                r = _process_ntff_profile(
                    profile, neff_dir, _COMPILED["nc"], list(range(NCORES)),
                    list(range(NCORES)), False, {}, trace_events=False,
                )
                _COMPILED["exec_time_ns"] = r.exec_time_ns
                _COMPILED["mean_exec_time_ns"] = r.mean_exec_time_ns
            except Exception as e:
                _COMPILED["exec_time_ns"] = None
                _COMPILED["trace_err"] = repr(e)
    else:
        results = run(in_maps)
    return _assemble(results)
